# revision 2
# baseline (speedup 1.0000x reference)
"""DLRM dot-interaction v3: bf16, xbar DMA transpose, 2-sample matmuls.

Per group of 128 samples:
  1. SWDGE cast-DMA (gpsimd): feats rows -> A[b, 128f + d] bf16 (sample on
     partition, 13.8KB contiguous per partition).
  2. ACT HWDGE xbar dma_start_transpose: A -> XT[d, 128f + b] bf16
     (feature-major blocks), one DMA per group. No PE transposes, no psum
     XT copies.
  3. PE: 64 two-sample bf16 matmuls (2 sheets of 32), lhsT pair dim via
     stride-delta column APs:
       sheet H, tile P2 (64 rows), q=0..15:
         lb0 = 64*P2 + 16*H + q, delta = 48 (H=0) / 16 (H=1)
         samples {lb0 + delta*a'}, psum rows 64*P2 + 32*a'+i,
         psum col = 27*slot + j, slot = q + 16*a' (H=0) / q + 16*(1-a') (H=1)
     Off-diagonal garbage lands only in never-used (row-parity, slot-half)
     cells.
  4. DVE: per sheet one fused stream-transpose (nested AP) psum ->
     FLAT_H[32A + slot, 32j + i]; merge sheet-1's valid row runs
     (16:48, 80:112) into FLAT0 -> sample-major FLAT0[b, 32j+i] = G_b[i,j].
  5. gpsimd: 26 strided copies gather tril -> OUTT [128, 351].
  6. SP DMA: OUTT -> out[:, 128:479]; ACT DMA: bottom -> out[:, 0:128].
"""

import numpy as np

import concourse.bass as bass
import concourse.mybir as mybir

F = 27
D = 128
NPAIR = F * (F - 1) // 2  # 351
OUTW = D + NPAIR  # 479
MG = 128
N_CORES = 8
B_FULL = 65536
NB = B_FULL // N_CORES
AW = F * D  # 3456
XTW = 4096

FP32 = mybir.dt.float32
BF16 = mybir.dt.bfloat16


def tri(i):
    return i * (i - 1) // 2


def build_nc(nb=NB, repeat=1):
    assert nb % MG == 0
    n_data = nb // MG
    n_mg = n_data * repeat
    nc = bass.Bass()
    feats = nc.dram_tensor("features", [nb, F, D], FP32, kind="ExternalInput")
    bottom = nc.dram_tensor("bottom_mlp_out", [nb, D], FP32, kind="ExternalInput")
    out = nc.dram_tensor("out", [nb, OUTW], FP32, kind="ExternalOutput")

    feats_rows = feats[:].rearrange("b f d -> b (f d)")  # [nb, 3456] f32

    from contextlib import ExitStack

    with ExitStack() as ctx:
        sem = lambda n: ctx.enter_context(nc.semaphore(n))
        sbuf = lambda n, s, d: ctx.enter_context(nc.sbuf_tensor(n, s, d))
        ps = lambda n, s, d: ctx.enter_context(nc.psum_tensor(n, s, d))
        s_mm, s_tr, s_mg, s_ga, s_bot, s_pad = (
            sem("s_mm"), sem("s_tr"), sem("s_mg"), sem("s_ga"),
            sem("s_bot"), sem("s_pad"),
        )
        s_a = [sem("s_a0"), sem("s_a1")]
        s_x = [sem("s_x0"), sem("s_x1")]
        s_do = [sem("s_do0"), sem("s_do1")]
        A = [sbuf("A0", [128, AW], BF16), sbuf("A1", [128, AW], BF16)]
        XT = [sbuf("XT0", [128, XTW], BF16), sbuf("XT1", [128, XTW], BF16)]
        FLAT0 = [sbuf("FLAT0_0", [128, 864], FP32), sbuf("FLAT0_1", [128, 864], FP32)]
        OUTT = [sbuf("OUTT0", [128, NPAIR], FP32), sbuf("OUTT1", [128, NPAIR], FP32)]
        gs = [ps(f"gs{i}", [128, 1024], FP32) for i in range(2)]
        block = ctx.enter_context(nc.Block())

        def a_src(g):
            dg = g % n_data
            return feats_rows[MG * dg : MG * (dg + 1), :]

        @block.sync
        def _(sync):
            for h in range(n_mg):
                dh = h % n_data
                sync.wait_ge(s_ga, h + 1)
                sync.dma_start(
                    out[MG * dh : MG * (dh + 1), D:OUTW], OUTT[h % 2][:]
                ).then_inc(s_do[h % 2], 16)
            for sl in range(2):
                sync.wait_ge(s_do[sl], 16 * ((n_mg + 1 - sl) // 2))
            sync.wait_ge(s_bot, 16 * repeat)

        @block.tensor
        def _(tensor):
            tensor.wait_ge(s_pad, 2)
            for g in range(n_mg):
                sl = g % 2
                tensor.wait_ge(s_x[sl], 16 * (g // 2 + 1))
                if g >= 2:
                    tensor.wait_ge(s_tr, g - 1)  # gs[sl] WAR vs extraction
                for lb in range(128):
                    P, rem = divmod(lb, 32)
                    k, b1 = divmod(rem, 16)
                    c0 = 512 * k + F * b1
                    ins = nc.tensor.matmul(
                        gs[sl][32 * P : 32 * P + 32, c0 : c0 + F],
                        bass.AP(XT[sl], lb, [[XTW, 128], [128, 32]]),
                        bass.AP(XT[sl], lb, [[XTW, 128], [128, F]]),
                        start=True,
                        stop=True,
                        tile_position=(0, 32 * P),
                    )
                    if lb == 127:
                        ins.then_inc(s_mm, 1)

        @block.vector
        def _(vector):
            vector.memset(XT[0][:, AW:XTW], 0.0).then_inc(s_pad, 1)
            vector.memset(XT[1][:, AW:XTW], 0.0).then_inc(s_pad, 1)
            for g in range(n_mg):
                sl = g % 2
                vector.wait_ge(s_mm, g + 1)
                if g >= 2:
                    vector.wait_ge(s_ga, g - 1)  # FLAT0[sl] WAR vs gather
                in_ap = bass.AP(gs[sl], 0, [[1024, 128], [1, F], [512, 2], [F, 16]])
                vector.transpose(FLAT0[sl][:], in_ap).then_inc(s_tr, 1)

        @block.scalar
        def _(scalar):
            for rep in range(repeat):
                scalar.dma_start(out[:, 0:D], bottom[:, :]).then_inc(s_bot, 16)
            for g in range(n_mg):
                sl = g % 2
                scalar.wait_ge(s_a[sl], 16 * (g // 2 + 1))
                if g >= 2:
                    scalar.wait_ge(s_mm, g - 1)  # XT[sl] WAR vs MMs g-2
                xt_view = XT[sl][:, 0:AW].rearrange("p (c d) -> p c d", d=128)
                scalar.dma_start_transpose(xt_view, A[sl][:]).then_inc(s_x[sl], 16)

        @block.gpsimd
        def _(gpsimd):
            def gather(h):
                gpsimd.wait_ge(s_tr, h + 1)
                if h >= 2:
                    gpsimd.wait_ge(s_do[h % 2], 16 * (h // 2))
                for i in range(1, F):
                    src = bass.AP(FLAT0[h % 2], i, [[864, 128], [32, i]])
                    ins = gpsimd.tensor_copy(OUTT[h % 2][:, tri(i) : tri(i) + i], src)
                    if i == F - 1:
                        ins.then_inc(s_ga, 1)

            for g in range(n_mg):
                sl = g % 2
                if g >= 2:
                    gpsimd.wait_ge(s_x[sl], 16 * ((g - 2) // 2 + 1))  # A[sl] WAR
                gpsimd.dma_start(A[sl][:], a_src(g)).then_inc(s_a[sl], 16)
                if g >= 2:
                    gather(g - 2)
            for h in (n_mg - 2, n_mg - 1):
                if h >= 0:
                    gather(h)

    return nc


_NC_CACHE = {}


def _get_nc(nb):
    if nb not in _NC_CACHE:
        _NC_CACHE[nb] = build_nc(nb)
    return _NC_CACHE[nb]


def kernel(features: np.ndarray, bottom_mlp_out: np.ndarray) -> np.ndarray:
    from concourse.bass_utils import run_bass_kernel_spmd

    B = features.shape[0]
    nb = B // N_CORES
    nc = _get_nc(nb)
    features = np.ascontiguousarray(features, dtype=np.float32)
    bottom_mlp_out = np.ascontiguousarray(bottom_mlp_out, dtype=np.float32)
    in_maps = [
        {
            "features": features[i * nb : (i + 1) * nb],
            "bottom_mlp_out": bottom_mlp_out[i * nb : (i + 1) * nb],
        }
        for i in range(N_CORES)
    ]
    res = run_bass_kernel_spmd(nc, in_maps, core_ids=list(range(N_CORES)))
    return np.concatenate([r["out"] for r in res.results], axis=0)
